# revision 11
# baseline (speedup 1.0000x reference)
"""Trainium2 Bass kernel for the masked-relu multi-head attention module.

Math (per batch b):
    qh = relu(q @ Wq.T + bq); kh, vh likewise
    scores = (qh/sqrt(D)) @ kh.T + mask        [per head]
    attn   = relu(softmax(scores) + mask2)
    out    = relu((attn @ vh)_concat @ Wo.T + bo)

Sharding: 8 cores = (batch b in 0..1) x (query block qb in 0..3).
Each core handles 512 queries of one batch, all 16 heads, all 2048 keys.

Device-side layout trick: scores are computed TRANSPOSED, [keys_part,
queries_free], which makes both attention matmuls transpose-free:
  scoresT = khT_chunk.T-as-lhsT @ qhT       (both [dim, token] layouts)
  outT    = vh-as-lhsT @ attn_T             (vh natural [token, dim])
The softmax denominator (a partition-axis sum in this layout) comes from a
ones-vector matmul on the PE. All host-side work is pure layout (transpose /
slice / concat) - every FLOP of the module runs on device.
"""

import os
import sys

sys.path.insert(0, "/opt/trn_rl_repo")

import numpy as np

from concourse import mybir
import concourse.bass as bass
import concourse.tile as tile
from concourse import bacc
from concourse.bass import ds, ts
from concourse.bass_utils import run_bass_kernel_spmd

B, S, E, H, D = 2, 2048, 1024, 16, 64
NCORES = 8
QB = NCORES // B            # query blocks per batch
NQ = S // QB                # queries per core (512)
P = 128
EC = E // P                 # 8 e-chunks
TC = S // P                 # 16 key chunks
SCALE = 1.0 / 8.0           # 1/sqrt(D)

F32 = mybir.dt.float32
BF16 = mybir.dt.bfloat16


def _emit(tc, io):
    """Emit the per-core program. io: dict of DRAM APs."""
    from contextlib import ExitStack

    nc = tc.nc
    Relu = mybir.ActivationFunctionType.Relu
    Exp = mybir.ActivationFunctionType.Exp
    Alu = mybir.AluOpType

    with ExitStack() as ctx:
        # ---------------- constants ----------------
        cpool = ctx.enter_context(tc.tile_pool(name="const", bufs=1))
        ones128 = cpool.tile([P, 1], BF16)
        nc.vector.memset(ones128[:], 1.0)
        ones1 = cpool.tile([1, P], F32)
        nc.vector.memset(ones1[:], 1.0)

        bq_t = cpool.tile([P, EC], F32)
        nc.sync.dma_start(bq_t[:], io["bq"].rearrange("(j p) -> p j", p=P))
        bk_t = cpool.tile([P, EC], F32)
        nc.sync.dma_start(bk_t[:], io["bk"].rearrange("(j p) -> p j", p=P))
        bo_t = cpool.tile([P, EC], F32)
        nc.sync.dma_start(bo_t[:], io["bo"].rearrange("(j p) -> p j", p=P))
        bv_t = cpool.tile([1, E], F32)
        nc.sync.dma_start(bv_t[:], io["bv"].rearrange("(o e) -> o e", o=1))

        # long-lived activations
        qpool = ctx.enter_context(tc.tile_pool(name="qh", bufs=1))
        qhT = qpool.tile([P, EC, NQ], F32)          # [128, 8, 512] = 2MB
        hpool = ctx.enter_context(tc.tile_pool(name="headcat", bufs=1))
        headcat = hpool.tile([P, EC, NQ], F32)      # attn output, [dim, q]

        dram = ctx.enter_context(tc.tile_pool(name="dram", bufs=1, space="DRAM"))
        khT_d = dram.tile([E, S], F32)              # [dim, tokens]
        vh_d = dram.tile([S, E], F32)               # [tokens, dim]

        # ---------------- projections ----------------
        with tc.tile_pool(name="wt", bufs=2) as wpool, \
             tc.tile_pool(name="xt", bufs=2) as xpool, \
             tc.tile_pool(name="pps", bufs=4, space="PSUM") as ppsum, \
             tc.tile_pool(name="pout", bufs=4) as opool:

            def load_w(name):
                w_t = wpool.tile([P, EC, E], F32, tag="w")
                nc.sync.dma_start(w_t[:], io[name].rearrange("(eo p) d -> p eo d", p=P))
                return w_t

            # q projection -> qhT resident [dim, 512]
            wq_t = load_w("wqT")
            x_t = xpool.tile([P, EC, NQ], F32, tag="x")
            nc.sync.dma_start(x_t[:], io["qT"].rearrange("(eo p) t -> p eo t", p=P))
            for j in range(EC):
                ps = ppsum.tile([P, NQ], F32, tag="ps")
                for e in range(EC):
                    nc.tensor.matmul(ps[:], wq_t[:, e, ts(j, P)], x_t[:, e, :],
                                     start=(e == 0), stop=(e == EC - 1))
                nc.scalar.activation(qhT[:, j, :], ps[:], Relu, bias=bq_t[:, ds(j, 1)])

            # k projection -> khT_d [dim, 2048] in DRAM
            wk_t = load_w("wkT")
            for tb in range(S // NQ):
                x_t = xpool.tile([P, EC, NQ], F32, tag="x")
                nc.sync.dma_start(
                    x_t[:], io["kT"].rearrange("(eo p) t -> p eo t", p=P)[:, :, ts(tb, NQ)])
                for j in range(EC):
                    ps = ppsum.tile([P, NQ], F32, tag="ps")
                    for e in range(EC):
                        nc.tensor.matmul(ps[:], wk_t[:, e, ts(j, P)], x_t[:, e, :],
                                         start=(e == 0), stop=(e == EC - 1))
                    o_t = opool.tile([P, NQ], F32, tag="o")
                    nc.scalar.activation(o_t[:], ps[:], Relu, bias=bk_t[:, ds(j, 1)])
                    nc.sync.dma_start(
                        khT_d[:].rearrange("(jo p) t -> p jo t", p=P)[:, j, ts(tb, NQ)],
                        o_t[:])

            # v projection -> vh_d [2048, dim] in DRAM (natural layout).
            # out[t_chunk, d] = sum_e vT[e, t].T @ WvT[e, d]; bias via ones-row
            # rank-1 matmul (bias is along the free axis here).
            wv_t = load_w("wvT")
            for tb in range(S // NQ):
                x_t = xpool.tile([P, EC, NQ], F32, tag="x")
                nc.sync.dma_start(
                    x_t[:], io["vT"].rearrange("(eo p) t -> p eo t", p=P)[:, :, ts(tb, NQ)])
                for tc2 in range(NQ // P):          # 4 token chunks of 128
                    for n in range(E // NQ):        # 2 output-dim halves of 512
                        ps = ppsum.tile([P, NQ], F32, tag="ps")
                        for e in range(EC):
                            nc.tensor.matmul(ps[:], x_t[:, e, ts(tc2, P)],
                                             wv_t[:, e, ts(n, NQ)],
                                             start=(e == 0), stop=False)
                        nc.tensor.matmul(ps[:], ones1[:], bv_t[:, ts(n, NQ)],
                                         start=False, stop=True)
                        o_t = opool.tile([P, NQ], F32, tag="o")
                        nc.scalar.activation(o_t[:], ps[:], Relu)
                        nc.sync.dma_start(
                            vh_d[ds(tb * NQ + tc2 * P, P), ts(n, NQ)], o_t[:])

        # ---------------- attention ----------------
        with tc.tile_pool(name="mask", bufs=1) as mpool, \
             tc.tile_pool(name="kv", bufs=1) as kvpool, \
             tc.tile_pool(name="p", bufs=2) as ppool, \
             tc.tile_pool(name="work", bufs=2) as wk, \
             tc.tile_pool(name="invd", bufs=1) as ivpool, \
             tc.tile_pool(name="sps", bufs=2, space="PSUM") as spsum, \
             tc.tile_pool(name="dps", bufs=1, space="PSUM") as dpsum, \
             tc.tile_pool(name="ops", bufs=1, space="PSUM") as opsum, \
             tc.tile_pool(name="bps", bufs=1, space="PSUM") as bpsum, \
             tc.tile_pool(name="dbounce", bufs=2, space="DRAM") as dbp:

            maskT_t = mpool.tile([P, TC, NQ], F32)   # 4MB
            nc.sync.dma_start(maskT_t[:], io["maskT"].rearrange("(c p) q -> p c q", p=P))
            m2T_t = mpool.tile([P, TC, NQ], F32)     # 4MB
            nc.sync.dma_start(m2T_t[:], io["mask2T"].rearrange("(c p) q -> p c q", p=P))

            for pair in range(H // 2):               # two heads per 128-row block
                khT_pair = kvpool.tile([P, S], F32, tag="kh")
                nc.sync.dma_start(
                    khT_pair[:], khT_d[:].rearrange("(jo p) t -> p jo t", p=P)[:, pair, :])
                vh_pair = kvpool.tile([P, TC, P], F32, tag="vh")
                nc.sync.dma_start(
                    vh_pair[:],
                    vh_d[:].rearrange("(c p) d -> p c d", p=P)[:, :, ts(pair, P)])

                p_t = [ppool.tile([P, TC, NQ], BF16, tag=f"p{hh}", name=f"p{hh}")
                       for hh in range(2)]
                spart = [ds(0, D), ds(D, D)]

                # scores + exp; the two heads use disjoint PE row groups
                for c in range(TC):
                    for hh in range(2):
                        s_ps = spsum.tile([P, NQ], F32, tag="s")
                        nc.tensor.matmul(s_ps[:], khT_pair[spart[hh], ts(c, P)],
                                         qhT[spart[hh], pair, :], start=True, stop=True)
                        s1 = wk.tile([P, NQ], F32, tag="s1")
                        nc.vector.scalar_tensor_tensor(
                            s1[:], s_ps[:], SCALE, maskT_t[:, c, :],
                            op0=Alu.mult, op1=Alu.add)
                        nc.scalar.activation(p_t[hh][:, c, :], s1[:], Exp)

                # softmax denominators via ones-matmul, then 1/d broadcast
                invd_b = []
                for hh in range(2):
                    d_ps = dpsum.tile([1, NQ], F32, tag=f"d{hh}")
                    for c in range(TC):
                        nc.tensor.matmul(d_ps[:], ones128[:], p_t[hh][:, c, :],
                                         start=(c == 0), stop=(c == TC - 1))
                    d_sb = ivpool.tile([1, NQ], F32, tag="dsb", name=f"dsb{hh}")
                    nc.vector.tensor_copy(d_sb[:], d_ps[:])
                    d_dram = dbp.tile([NQ], F32, tag=f"dd{hh}")
                    nc.sync.dma_start(d_dram[:].rearrange("(o q) -> o q", o=1), d_sb[:])
                    d_r = ivpool.tile([P, NQ // P], F32, tag="dr")
                    nc.sync.dma_start(d_r[:], d_dram[:].rearrange("(p f) -> p f", p=P))
                    iv_r = ivpool.tile([P, NQ // P], F32, tag="ivr")
                    nc.vector.reciprocal(iv_r[:], d_r[:])
                    iv_dram = dbp.tile([NQ], F32, tag="ivd")
                    nc.sync.dma_start(iv_dram[:].rearrange("(p f) -> p f", p=P), iv_r[:])
                    iv_f = ivpool.tile([1, NQ], F32, tag="ivf")
                    nc.sync.dma_start(iv_f[:], iv_dram[:].rearrange("(o q) -> o q", o=1))
                    b_ps = bpsum.tile([P, NQ], F32, tag="b")
                    nc.tensor.matmul(b_ps[:], ones1[:], iv_f[:], start=True, stop=True)
                    ib = ivpool.tile([P, NQ], F32, tag=f"ib{hh}")
                    nc.vector.tensor_copy(ib[:], b_ps[:])
                    invd_b.append(ib)

                # attn = relu(p/d + mask2); outT_h += vh_chunk.T-as-lhsT @ attn
                o_ps = [opsum.tile([D, NQ], F32, tag=f"o{hh}", name=f"ops{hh}")
                        for hh in range(2)]
                for c in range(TC):
                    for hh in range(2):
                        p2 = wk.tile([P, NQ], F32, tag="p2")
                        nc.vector.tensor_mul(p2[:], p_t[hh][:, c, :], invd_b[hh][:])
                        w_t = wk.tile([P, NQ], F32, tag="w")
                        nc.vector.tensor_add(w_t[:], p2[:], m2T_t[:, c, :])
                        nc.vector.tensor_scalar_max(w_t[:], w_t[:], 0.0)
                        nc.tensor.matmul(o_ps[hh][:], vh_pair[:, c, spart[hh]], w_t[:],
                                         start=(c == 0), stop=(c == TC - 1))
                for hh in range(2):
                    h = 2 * pair + hh
                    nc.scalar.copy(headcat[spart[hh], pair, :], o_ps[hh][:])

        # ---------------- output projection ----------------
        with tc.tile_pool(name="wo", bufs=1) as wopool, \
             tc.tile_pool(name="ops2", bufs=4, space="PSUM") as opsum2, \
             tc.tile_pool(name="oout", bufs=4) as oopool:
            wo_t = wopool.tile([P, EC, E], F32)
            nc.sync.dma_start(wo_t[:], io["woT"].rearrange("(eo p) d -> p eo d", p=P))
            for j in range(EC):
                ps = opsum2.tile([P, NQ], F32, tag="ps")
                for e in range(EC):
                    nc.tensor.matmul(ps[:], wo_t[:, e, ts(j, P)], headcat[:, e, :],
                                     start=(e == 0), stop=(e == EC - 1))
                o_t = oopool.tile([P, NQ], F32, tag="o")
                nc.scalar.activation(o_t[:], ps[:], Relu, bias=bo_t[:, ds(j, 1)])
                nc.sync.dma_start(
                    io["outT"].rearrange("(jo p) q -> p jo q", p=P)[:, j, :], o_t[:])


_PROGRAM = None


def _build_program():
    global _PROGRAM
    if _PROGRAM is not None:
        return _PROGRAM
    nc = bacc.Bacc("TRN2", target_bir_lowering=False, debug=False,
                   num_devices=NCORES)
    io = {}
    def inp(name, shape):
        io[name] = nc.dram_tensor(name, shape, F32, kind="ExternalInput").ap()
    inp("qT", [E, NQ])
    inp("kT", [E, S])
    inp("vT", [E, S])
    inp("maskT", [S, NQ])
    inp("mask2T", [S, NQ])
    for w in ("wqT", "wkT", "wvT", "woT"):
        inp(w, [E, E])
    for b in ("bq", "bk", "bv", "bo"):
        inp(b, [E])
    io["outT"] = nc.dram_tensor("outT", [E, NQ], F32, kind="ExternalOutput").ap()

    with tile.TileContext(nc) as tc:
        _emit(tc, io)
    nc.compile()
    _PROGRAM = (nc, io)
    return _PROGRAM


def kernel(q, k, v, mask, mask2, Wq, bq, Wk, bk, Wv, bv, Wo, bo, _trace=False):
    nc, _ = _build_program()

    f = np.float32
    wqT = np.ascontiguousarray(Wq.T, dtype=f)
    wkT = np.ascontiguousarray(Wk.T, dtype=f)
    wvT = np.ascontiguousarray(Wv.T, dtype=f)
    woT = np.ascontiguousarray(Wo.T, dtype=f)

    in_maps = []
    for c in range(NCORES):
        b, qb = divmod(c, QB)
        rows = slice(qb * NQ, (qb + 1) * NQ)
        in_maps.append({
            "qT": np.ascontiguousarray(q[b, rows, :].T, dtype=f),
            "kT": np.ascontiguousarray(k[b].T, dtype=f),
            "vT": np.ascontiguousarray(v[b].T, dtype=f),
            "maskT": np.ascontiguousarray(mask[b, rows, :].T, dtype=f),
            "mask2T": np.ascontiguousarray(mask2[b, rows, :].T, dtype=f),
            "wqT": wqT, "wkT": wkT, "wvT": wvT, "woT": woT,
            "bq": np.ascontiguousarray(bq, dtype=f),
            "bk": np.ascontiguousarray(bk, dtype=f),
            "bv": np.ascontiguousarray(bv, dtype=f),
            "bo": np.ascontiguousarray(bo, dtype=f),
        })

    res = run_bass_kernel_spmd(nc, in_maps, core_ids=list(range(NCORES)),
                               trace=_trace)

    out = np.empty((B, S, E), dtype=f)
    for c in range(NCORES):
        b, qb = divmod(c, QB)
        out[b, qb * NQ:(qb + 1) * NQ, :] = res.results[c]["outT"].T
    if _trace:
        kernel.last_results = res
    return out
